# revision 36
# baseline (speedup 1.0000x reference)
"""Trainium2 Bass kernel: dimensional (channel) attention token-mixing block.

Computation (per batch b):
    xt = x[b].T                               # [C, N]
    q  = xt @ wq.T                            # [C, N]   (heads: N = H*NH)
    kv = xt @ wkv.T                           # [C, 2N]
    q, k normalized over NH per (c, head)
    kt[h] = sum_c k_hat[c,h,:] * v[c,h,:] * scale[h]     # [NH]
    o  = gelu(q_hat * kt)                     # [C, N]
    out[b] = (o @ wproj.T + bproj).T          # [N, C]

Sharding: data-parallel over B across 8 cores (2 batches/core), weights
replicated.  All tensors are kept in "transposed" [n, c] layout on device so
the contraction dim n always lies on SBUF partitions:
  - stage 1: out_psum[m, c] += wT[n_chunk, m].T @ x[n_chunk, c]
  - q/k sum-of-squares over m (partitions) via gpsimd partition_all_reduce
    (keeps the PE free for the main matmuls)
  - kernel trick (reduction over c = free dim) via tensor_mul + tensor_reduce
  - stage 2: out[m, c] += wprojT[n_chunk, m].T @ o[n_chunk, c]
Weights are transposed + cast to bf16 on the host (fp32 cannot DMA-transpose,
and bf16 matmuls run 4x faster than fp32 on the PE).  PSUM accumulates fp32.

Schedule: DMA transfers serialize at ~332GB/s, so everything streams on the
SP HWDGE queue in an explicitly interleaved order (first q-tile weights and
x chunks alternating; sub-0 weights arrive as quarter tiles).  The first
four q output tiles run chunk-major over all 8 PSUM banks so the PE starts
~4.2us in and stays busy while the 12.6MB x tensor streams; zero-dependency
warm-up matmuls keep the PE clock ramped through the DMA-paced waits.  The
final projection tile runs as six staggered third-column accumulation
groups so only one third-width bias->DMA drain chain (~3.7us) remains after
the last matmul.
"""

import sys

import numpy as np
import ml_dtypes

if "/opt/trn_rl_repo" not in sys.path:
    sys.path.insert(0, "/opt/trn_rl_repo")

import concourse.bass as bass
import concourse.bacc as bacc
import concourse.bass_isa as bass_isa
import concourse.mybir as mybir
import concourse.tile as tile
from concourse.bass_utils import run_bass_kernel_spmd

B, N, C, H = 16, 4096, 384, 8
NH = N // H          # 512
P = 128
NT = N // P          # 32 contraction chunks of 128
MSUB = NH // P       # 4 output row-subtiles per head
NCORES = 8
BPC = B // NCORES    # batches per core

dt = mybir.dt
AF = mybir.ActivationFunctionType
ALU = mybir.AluOpType
BF16 = ml_dtypes.bfloat16

_NC = None
_LAST_RESULTS = None


def _build_nc(act_fn=None):
    if act_fn is None:
        act_fn = AF.Gelu
    nc = bacc.Bacc("TRN2", target_bir_lowering=False, debug=False)

    x_d = nc.dram_tensor("x", [P, NT, BPC, C], dt.bfloat16, kind="ExternalInput")
    wq_d = nc.dram_tensor("wq", [NT, P, NT, P], dt.bfloat16, kind="ExternalInput")
    wkv_d = nc.dram_tensor("wkv", [2 * NT, P, NT, P], dt.bfloat16, kind="ExternalInput")
    wp_d = nc.dram_tensor("wproj", [NT, P, NT, P], dt.bfloat16, kind="ExternalInput")
    bias_d = nc.dram_tensor("bias", [P, NT], dt.float32, kind="ExternalInput")
    scale_d = nc.dram_tensor("scale", [P, H], dt.float32, kind="ExternalInput")
    out_d = nc.dram_tensor("out", [BPC, NT, P, C], dt.float32, kind="ExternalOutput")

    with tile.TileContext(nc) as tc:
        with (
            tc.tile_pool(name="const", bufs=1) as cpool,
            tc.tile_pool(name="wts", bufs=8) as wpool,
            tc.tile_pool(name="qkv", bufs=1) as qkvpool,
            tc.tile_pool(name="sqp", bufs=2) as sqpool,
            tc.tile_pool(name="nrm", bufs=2) as nrmpool,
            tc.tile_pool(name="scr", bufs=4) as scrpool,
            tc.tile_pool(name="outp", bufs=4) as outpool,
            tc.tile_pool(name="wq0", bufs=2) as wq0pool,
            tc.tile_pool(name="mmps", bufs=8, space="PSUM") as mmpsum,
        ):
            x_sb = cpool.tile([P, NT, BPC, C], dt.bfloat16)
            o_sb = cpool.tile([P, NT, BPC, C], dt.bfloat16)
            scale_sb = cpool.tile([P, H], dt.float32)
            bias_sb = cpool.tile([P, NT], dt.float32)

            # Weight tiles are split in half and streamed through an 8-deep
            # pool: the slot-reuse (WAW) partner is then exactly 8 HWDGE DMAs
            # back, which lands on the same DMAHW queue semaphore — implicit
            # FIFO ordering instead of an extra sync wait (walrus allows only
            # 2 waits per DMA instruction).
            NTH = NT // 2

            # ---- startup: x load + q head-0 sub 0..3, chunk-major ----
            # DMA transfers serialize on one engine-model resource, so the SP
            # queue order below IS the arrival order.  The first four q tiles
            # run chunk-major over 8 concurrent PSUM groups (4 subs x 2
            # batches) so the PE starts as soon as (w0-half0, x-chunk0) land
            # and never waits for the full 12.6MB x tensor.
            sg_w = {}
            NTQ = NT // 4

            def sg_w_dma(sub, half):
                t = wpool.tile([P, NTH, P], dt.bfloat16, tag="w", name="w_sb")
                nc.sync.dma_start(
                    t[:], wq_d[sub, :, half * NTH : (half + 1) * NTH]
                )
                sg_w[(sub, half)] = t

            def sg_wq0_dma(quarter):
                # sub-0's h0 half comes as two quarter tiles so the very
                # first stationary lands ~0.7us earlier
                t = wq0pool.tile([P, NTQ, P], dt.bfloat16, tag="wq", name="wq0")
                nc.sync.dma_start(
                    t[:], wq_d[0, :, quarter * NTQ : (quarter + 1) * NTQ]
                )
                sg_w[("q", quarter)] = t

            def x_dma(kt0, n):
                nc.sync.dma_start(
                    x_sb[:, kt0 : kt0 + n], x_d[:, kt0 : kt0 + n]
                )

            sg_wq0_dma(0); x_dma(0, 1); x_dma(1, 1)
            sg_w_dma(1, 0); x_dma(2, 2); x_dma(4, 2)
            sg_w_dma(2, 0); sg_wq0_dma(1); x_dma(6, 2); x_dma(8, 2)
            sg_w_dma(3, 0); x_dma(10, 2); x_dma(12, 2); x_dma(14, 2)
            sg_w_dma(0, 1); x_dma(16, 2); x_dma(18, 2)
            sg_w_dma(1, 1); x_dma(20, 2); x_dma(22, 2)
            sg_w_dma(2, 1); x_dma(24, 2); x_dma(26, 2)
            sg_w_dma(3, 1); x_dma(28, 2); x_dma(30, 2)
            nc.sync.dma_start(scale_sb[:], scale_d[:])
            nc.sync.dma_start(bias_sb[:], bias_d[:])

            # PE p-state warm-up: the tensor engine clock ramps over ~3us of
            # continuous execution.  Zero-dependency dummy matmuls keep the
            # PE busy while the first w/x DMAs land, so the real matmuls run
            # at full clock from the start.  The warm psum tile's bank is
            # shared (pool slot cycling) with ps_sg[3][1], whose first write
            # is PE-ordered after the last dummy.
            warm_sb = cpool.tile([P, 64], dt.bfloat16)
            nc.gpsimd.memset(warm_sb[:], 1.0)
            warm_ps = mmpsum.tile([64, 64], dt.float32, tag="mm", name="warm")

            def warm(n):
                for _ in range(n):
                    nc.tensor.matmul(warm_ps[:], warm_sb[:], warm_sb[:])

            warm(56)

            ps_sg = [
                [
                    mmpsum.tile([P, C], dt.float32, tag="mm", name="ps")
                    for _ in range(BPC)
                ]
                for _ in range(MSUB)
            ]
            # emission order matched to the DMA arrival order above: each
            # entry is (sub, kt list); a sub joins once its h0 half landed
            # and catches up on all prior chunks.
            SG_ORDER = [
                (0, [0]), (0, [1]), (1, [0, 1]),
                (0, [2, 3]), (1, [2, 3]), (0, [4, 5]), (1, [4, 5]),
                (2, [0, 1, 2, 3, 4, 5]),
                (0, [6, 7]), (1, [6, 7]), (2, [6, 7]),
                (0, [8, 9]), (1, [8, 9]), (2, [8, 9]),
                (3, [0, 1, 2, 3, 4, 5, 6, 7, 8, 9]),
                (0, [10, 11]), (1, [10, 11]), (2, [10, 11]), (3, [10, 11]),
                (0, [12, 13]), (1, [12, 13]), (2, [12, 13]), (3, [12, 13]),
                (0, [14, 15]), (1, [14, 15]), (2, [14, 15]), (3, [14, 15]),
                (0, [16, 17]), (0, [18, 19]), (1, [16, 17, 18, 19]),
                (0, [20, 21]), (1, [20, 21]), (0, [22, 23]), (1, [22, 23]),
                (2, [16, 17, 18, 19, 20, 21, 22, 23]),
                (0, [24, 25]), (1, [24, 25]), (2, [24, 25]),
                (0, [26, 27]), (1, [26, 27]), (2, [26, 27]),
                (3, [16, 17, 18, 19, 20, 21, 22, 23, 24, 25, 26, 27]),
                (0, [28, 29]), (1, [28, 29]), (2, [28, 29]), (3, [28, 29]),
                (0, [30, 31]), (1, [30, 31]), (2, [30, 31]), (3, [30, 31]),
            ]
            for i, (sub, kts) in enumerate(SG_ORDER):
                for kt_ in kts:
                    if sub == 0 and kt_ < NTH:
                        w_sb = sg_w[("q", kt_ // NTQ)]
                        w_ap = w_sb[:, kt_ % NTQ, :]
                    else:
                        w_sb = sg_w[(sub, kt_ // NTH)]
                        w_ap = w_sb[:, kt_ % NTH, :]
                    for b in range(BPC):
                        nc.tensor.matmul(
                            ps_sg[sub][b][:],
                            w_ap,
                            x_sb[:, kt_, b, :],
                            start=(kt_ == 0),
                            stop=(kt_ == NT - 1),
                        )
                # fill the DMA-paced waits early in the supergroup so the
                # PE never idles (an idle resets the p-state ramp)
                if i == 1:
                    warm(22)
                elif i in (2, 3):
                    warm(10)

            def mm_tile(wsrc, widx, dst, dst_sub, src_sb):
                """One 128-wide output tile: psum[b] += wT_chunk.T @ src_chunk."""
                whalves = []
                for half in range(2):
                    w_sb = wpool.tile([P, NTH, P], dt.bfloat16, tag="w", name="w_sb")
                    nc.sync.dma_start(
                        w_sb[:], wsrc[widx, :, half * NTH : (half + 1) * NTH]
                    )
                    whalves.append(w_sb)
                ps = [
                    mmpsum.tile([P, C], dt.float32, tag="mm", name="ps")
                    for _ in range(BPC)
                ]
                for kt_ in range(NT):
                    w_sb = whalves[kt_ // NTH]
                    for b in range(BPC):
                        nc.tensor.matmul(
                            ps[b][:],
                            w_sb[:, kt_ % NTH, :],
                            src_sb[:, kt_, b, :],
                            start=(kt_ == 0),
                            stop=(kt_ == NT - 1),
                        )
                for b in range(BPC):
                    nc.any.tensor_copy(out=dst[:, dst_sub, b, :], in_=ps[b][:])

            # ---------------- stage 1: q/kv + attention + gelu ----------------
            for h in range(H):
                q_sb = qkvpool.tile([P, MSUB, BPC, C], dt.bfloat16, tag="q", name="q_sb")
                k_sb = qkvpool.tile([P, MSUB, BPC, C], dt.bfloat16, tag="k", name="k_sb")
                v_sb = qkvpool.tile([P, MSUB, BPC, C], dt.bfloat16, tag="v", name="v_sb")

                if h == 0:
                    # q computed by the chunk-major startup supergroup above
                    for sub in range(MSUB):
                        for b in range(BPC):
                            nc.any.tensor_copy(
                                out=q_sb[:, sub, b, :], in_=ps_sg[sub][b][:]
                            )
                else:
                    for sub in range(MSUB):
                        mm_tile(wq_d, h * MSUB + sub, q_sb, sub, x_sb)
                for sub in range(MSUB):
                    mm_tile(wkv_d, h * MSUB + sub, k_sb, sub, x_sb)
                for sub in range(MSUB):
                    mm_tile(wkv_d, NT + h * MSUB + sub, v_sb, sub, x_sb)

                for b in range(BPC):
                    # rnorm = 1/||.|| over the NH dim (4 sub-tiles x 128
                    # partitions).  Squares + pairwise adds on DVE, then the
                    # partition reduction on gpsimd (all partitions receive
                    # the sum), keeping the PE free for the main matmuls.
                    rnorms = {}
                    for which, src in (("q", q_sb), ("k", k_sb)):
                        sq = sqpool.tile([P, MSUB, C], dt.bfloat16, tag="sq", name="sq")
                        for sub in range(MSUB):
                            nc.vector.tensor_mul(
                                sq[:, sub, :], src[:, sub, b, :], src[:, sub, b, :]
                            )
                        acc0 = sqpool.tile([P, C], dt.float32, tag="acc0", name="acc0")
                        acc1 = sqpool.tile([P, C], dt.float32, tag="acc1", name="acc1")
                        nc.vector.tensor_add(acc0[:], sq[:, 0, :], sq[:, 1, :])
                        nc.vector.tensor_add(acc1[:], sq[:, 2, :], sq[:, 3, :])
                        ssq = sqpool.tile([P, C], dt.float32, tag="ssq", name="ssq")
                        nc.vector.tensor_add(ssq[:], acc0[:], acc1[:])
                        ssq_r = nrmpool.tile([P, C], dt.float32, tag="ssqr", name="ssq_r")
                        nc.gpsimd.partition_all_reduce(
                            ssq_r[:], ssq[:], channels=P, reduce_op=bass_isa.ReduceOp.add
                        )
                        issq = nrmpool.tile([P, C], dt.float32, tag="issq", name="issq")
                        nc.vector.reciprocal(issq[:], ssq_r[:])
                        rn = nrmpool.tile([P, C], dt.float32, tag=f"rn{which}", name="rn")
                        nc.scalar.sqrt(rn[:], issq[:])
                        rnorms[which] = rn

                    # kernel trick: kt[m] = scale[h] * sum_c k_hat[m,c]*v[m,c]
                    kt_sb = nrmpool.tile([P, MSUB], dt.float32, tag="kt", name="kt_sb")
                    for sub in range(MSUB):
                        vrk = scrpool.tile([P, C], dt.float32, tag="vrk", name="vrk")
                        nc.vector.tensor_mul(vrk[:], v_sb[:, sub, b, :], rnorms["k"][:])
                        prod = scrpool.tile([P, C], dt.float32, tag="prod", name="prod")
                        nc.vector.tensor_mul(prod[:], vrk[:], k_sb[:, sub, b, :])
                        nc.vector.tensor_reduce(
                            kt_sb[:, sub : sub + 1],
                            prod[:],
                            axis=mybir.AxisListType.X,
                            op=ALU.add,
                        )
                    nc.vector.tensor_scalar_mul(
                        kt_sb[:], kt_sb[:], scale_sb[:, h : h + 1]
                    )

                    # o = gelu(q * rnorm_q * kt)
                    for sub in range(MSUB):
                        gin = scrpool.tile([P, C], dt.float32, tag="gin", name="gin")
                        nc.vector.scalar_tensor_tensor(
                            out=gin[:],
                            in0=q_sb[:, sub, b, :],
                            scalar=kt_sb[:, sub : sub + 1],
                            in1=rnorms["q"][:],
                            op0=ALU.mult,
                            op1=ALU.mult,
                        )
                        nc.scalar.activation(
                            o_sb[:, h * MSUB + sub, b, :], gin[:], act_fn
                        )

            # ---------------- stage 2: output projection ----------------
            for mt in range(NT):
                whalves = []
                for half in range(2):
                    w_sb = wpool.tile([P, NTH, P], dt.bfloat16, tag="w", name="w_sb")
                    nc.sync.dma_start(
                        w_sb[:], wp_d[mt, :, half * NTH : (half + 1) * NTH]
                    )
                    whalves.append(w_sb)
                if mt < NT - 1:
                    ps = [
                        mmpsum.tile([P, C], dt.float32, tag="mm", name="ps")
                        for _ in range(BPC)
                    ]
                    for kt_ in range(NT):
                        w_sb = whalves[kt_ // NTH]
                        for b in range(BPC):
                            nc.tensor.matmul(
                                ps[b][:],
                                w_sb[:, kt_ % NTH, :],
                                o_sb[:, kt_, b, :],
                                start=(kt_ == 0),
                                stop=(kt_ == NT - 1),
                            )
                    for b in range(BPC):
                        ob = outpool.tile([P, C], dt.float32, tag="ob", name="ob")
                        nc.vector.tensor_scalar_add(
                            ob[:], ps[b][:], bias_sb[:, mt : mt + 1]
                        )
                        nc.sync.dma_start(out_d[b, mt], ob[:])
                else:
                    # drain tail: the final tile runs as six sequential
                    # third-column accumulation groups (each in its own psum
                    # bank), so each group's bias -> DMA chain overlaps the
                    # next group's matmuls and only the last third-width
                    # chain remains after the final matmul.  The last group
                    # biases on Act so its DMA issues on the same engine
                    # with no cross-engine semaphore hop.
                    CH = C // 3
                    for b in range(BPC):
                        for col in range(3):
                            cs = col * CH
                            pst = mmpsum.tile(
                                [P, C], dt.float32, tag="mm", name="ps"
                            )[:, :CH]
                            for kt_ in range(NT):
                                w_sb = whalves[kt_ // NTH]
                                nc.tensor.matmul(
                                    pst,
                                    w_sb[:, kt_ % NTH, :],
                                    o_sb[:, kt_, b, cs : cs + CH],
                                    start=(kt_ == 0),
                                    stop=(kt_ == NT - 1),
                                )
                            ob = outpool.tile(
                                [P, CH], dt.float32, tag="obh", name="obh"
                            )
                            if col == 2:
                                nc.scalar.activation(
                                    ob[:],
                                    pst,
                                    AF.Identity,
                                    bias=bias_sb[:, mt : mt + 1],
                                )
                                nc.scalar.dma_start(
                                    out_d[b, mt, :, cs : cs + CH], ob[:]
                                )
                            else:
                                nc.vector.tensor_scalar_add(
                                    ob[:], pst, bias_sb[:, mt : mt + 1]
                                )
                                nc.sync.dma_start(
                                    out_d[b, mt, :, cs : cs + CH], ob[:]
                                )

    nc.compile()
    return nc


def _prep_inputs(x, wq, wkv, wproj, bproj, scale):
    x = np.asarray(x, dtype=np.float32)
    wq = np.asarray(wq, dtype=np.float32)
    wkv = np.asarray(wkv, dtype=np.float32)
    wproj = np.asarray(wproj, dtype=np.float32)
    bproj = np.asarray(bproj, dtype=np.float32)
    scale = np.asarray(scale, dtype=np.float32)

    # W[mt, p, nt, j] = w[mt*128 + j, nt*128 + p]  (transposed tile layout)
    def wtiles(w, mtiles):
        return np.ascontiguousarray(
            w.reshape(mtiles, P, NT, P).transpose(0, 3, 2, 1)
        ).astype(BF16)

    Wq = wtiles(wq, NT)
    Wkv = wtiles(wkv, 2 * NT)
    Wp = wtiles(wproj, NT)

    # X[core][p, nt, b, c] = x[2*core + b, nt*128 + p, c]
    Xall = np.ascontiguousarray(
        x.reshape(NCORES, BPC, NT, P, C).transpose(0, 3, 2, 1, 4)
    ).astype(BF16)

    bias = np.ascontiguousarray(bproj.reshape(NT, P).T)
    scale_b = np.ascontiguousarray(
        np.broadcast_to(scale.reshape(1, H), (P, H))
    ).astype(np.float32)

    in_maps = []
    for c in range(NCORES):
        in_maps.append(
            {
                "x": Xall[c],
                "wq": Wq,
                "wkv": Wkv,
                "wproj": Wp,
                "bias": bias,
                "scale": scale_b,
            }
        )
    return in_maps


def kernel(x, wq, wkv, wproj, bproj, scale):
    global _NC, _LAST_RESULTS
    if _NC is None:
        _NC = _build_nc()

    in_maps = _prep_inputs(x, wq, wkv, wproj, bproj, scale)
    res = run_bass_kernel_spmd(_NC, in_maps, core_ids=list(range(NCORES)))
    _LAST_RESULTS = res

    outs = [res.results[c]["out"].reshape(BPC, N, C) for c in range(NCORES)]
    return np.ascontiguousarray(np.concatenate(outs, axis=0), dtype=np.float32)
